# revision 1
# baseline (speedup 1.0000x reference)
"""DGLJTNN decoder on 8 Trainium2 NeuronCores.

Data-parallel over the tree batch dim B=1024: each core handles 128 trees.
Chain-tree DFS schedule is static; the per-edge GRU recurrence runs 46
sequential steps on-device. Word-embedding projections (emb @ W_x parts) are
constant-folded on the host into per-vocab tables and gathered on-device by
word id. Hot GEMMs run in bf16 (weights+activations) with fp32 PSUM
accumulation; state updates, softmax and losses are fp32.
"""

import numpy as np
import ml_dtypes

import concourse.bass as bass
import concourse.mybir as mybir
import concourse.tile as tile
from concourse import bacc
from concourse.bass_utils import run_bass_kernel_spmd

F32 = mybir.dt.float32
BF16 = mybir.dt.bfloat16
I32 = mybir.dt.int32
AF = mybir.ActivationFunctionType
ALU = mybir.AluOpType

B, L, H, LAT, V = 1024, 24, 512, 128, 800
E = 2 * (L - 1)
T = E
NCORES = 8
BL = B // NCORES  # 128 trees per core
KH = H // 128     # 4 K-tiles of 128 over H

# ---- static DFS schedule (chain tree), mirrors reference.py ----
_f = np.arange(L - 1)
_b = np.arange(L - 2, -1, -1)
SRC = np.concatenate([_f, _b + 1])
DST = np.concatenate([_f + 1, _b])
# step semantics derived from the chain topology:
#   s/arm at step t = m/rm written at step t-1, except steps 0 and L-1 where
#   they are zero.  h_v at fwd step = m_new; at bwd step = m_new + m_fwd[v-1].
P_TGT_ROW = np.concatenate([np.ones(L - 1), np.zeros(L - 1), np.zeros(1)])  # 47 rows

_BF = ml_dtypes.bfloat16


def _build_nc(npos, with_q=True):
    nc = bacc.Bacc("TRN2", target_bir_lowering=False, debug=False, num_devices=NCORES)

    # ---------------- DRAM I/O ----------------
    d_wid = nc.dram_tensor("wid", [BL, L], I32, kind="ExternalInput")
    d_tv = nc.dram_tensor("tv", [BL, LAT], F32, kind="ExternalInput")
    # per-vocab projection tables (host folded: emb @ W_x + bias), bf16
    d_Pz = nc.dram_tensor("Pz", [V, H], BF16, kind="ExternalInput")
    d_Ph = nc.dram_tensor("Ph", [V, H], BF16, kind="ExternalInput")
    d_Pr = nc.dram_tensor("Pr", [V, H], BF16, kind="ExternalInput")
    d_Pu = nc.dram_tensor("Pu", [V, H], BF16, kind="ExternalInput")
    # recurrent / late weights, bf16, layout [128, K/128, N]
    d_Wzs = nc.dram_tensor("Wzs", [128, KH, H], BF16, kind="ExternalInput")
    d_Whm = nc.dram_tensor("Whm", [128, KH, H], BF16, kind="ExternalInput")
    d_Ur = nc.dram_tensor("Ur", [128, KH, H], BF16, kind="ExternalInput")
    d_Uh = nc.dram_tensor("Uh", [128, KH, H], BF16, kind="ExternalInput")
    d_Wwh = nc.dram_tensor("Wwh", [128, KH, H], BF16, kind="ExternalInput")
    d_Wo = nc.dram_tensor("Wo", [128, KH, V], BF16, kind="ExternalInput")
    d_Wutv = nc.dram_tensor("Wutv", [LAT, H], F32, kind="ExternalInput")
    d_Wwtv = nc.dram_tensor("Wwtv", [LAT, H], F32, kind="ExternalInput")
    d_bw = nc.dram_tensor("bwq", [128, KH], F32, kind="ExternalInput")
    d_bs = nc.dram_tensor("bs", [1, 1], F32, kind="ExternalInput")
    d_bo = nc.dram_tensor("bo", [1, V], F32, kind="ExternalInput")
    d_out = nc.dram_tensor("out", [1, 4], F32, kind="ExternalOutput")

    with tile.TileContext(nc) as tc:
        with (
            tc.tile_pool(name="persist", bufs=1) as pp,
            tc.tile_pool(name="ring", bufs=2) as rg,
            tc.tile_pool(name="ring1", bufs=1) as r1,
            tc.tile_pool(name="rgs", bufs=1) as rgs,
            tc.tile_pool(name="ps_rec", bufs=1, space="PSUM") as ps_rec,
            tc.tile_pool(name="ps_trm", bufs=2, space="PSUM") as ps_trm,
            tc.tile_pool(name="ps_trr", bufs=2, space="PSUM") as ps_trr,
        ):
            # ---------------- setup ----------------
            wid_s = pp.tile([BL, L], I32, tag="wid")
            nc.sync.dma_start(wid_s[:], d_wid[:])
            tv_s = pp.tile([BL, LAT], F32, tag="tv")
            nc.sync.dma_start(tv_s[:], d_tv[:])

            # weights
            w_Wzs = pp.tile([128, KH, H], BF16, tag="Wzs")
            w_Whm = pp.tile([128, KH, H], BF16, tag="Whm")
            w_Ur = pp.tile([128, KH, H], BF16, tag="Ur")
            w_Uh = pp.tile([128, KH, H], BF16, tag="Uh")
            w_Wwh = pp.tile([128, KH, H], BF16, tag="Wwh")
            w_Wo = pp.tile([128, KH, V], BF16, tag="Wo")
            for dst, src in [(w_Wzs, d_Wzs), (w_Whm, d_Whm), (w_Ur, d_Ur),
                             (w_Uh, d_Uh), (w_Wwh, d_Wwh), (w_Wo, d_Wo)]:
                nc.sync.dma_start(dst[:], src[:])
            w_Wutv = pp.tile([LAT, H], F32, tag="Wutv")
            nc.sync.dma_start(w_Wutv[:], d_Wutv[:])
            w_Wwtv = pp.tile([LAT, H], F32, tag="Wwtv")
            nc.sync.dma_start(w_Wwtv[:], d_Wwtv[:])
            bw_s = pp.tile([128, KH], F32, tag="bw")
            nc.sync.dma_start(bw_s[:], d_bw[:])
            bs_b = pp.tile([BL, 1], F32, tag="bs")
            nc.sync.dma_start(
                bs_b[:],
                bass.AP(tensor=d_bs, offset=0, ap=[[0, BL], [1, 1]]),
            )
            bo_f = rgs.tile([BL, V], F32, tag="mask")
            nc.sync.dma_start(
                bo_f[:],
                bass.AP(tensor=d_bo, offset=0, ap=[[0, BL], [1, V]]),
            )
            bo_b = pp.tile([BL, V], BF16, tag="bo")
            nc.scalar.copy(bo_b[:], bo_f[:])

            # iota row 0..V-1 (fp32, exact) + fp32 targets
            iota_f = pp.tile([BL, V], F32, tag="iota")
            nc.gpsimd.iota(iota_f[:], pattern=[[1, V]], base=0, channel_multiplier=0,
                           allow_small_or_imprecise_dtypes=True)
            tgtf = pp.tile([BL, L], F32, tag="tgtf")
            nc.vector.tensor_copy(tgtf[:], wid_s[:])

            # identities (fp32 + bf16) via iota compare
            pidx_f = r1.tile([128, 1], F32, tag="pidxf")
            nc.gpsimd.iota(pidx_f[:], pattern=[[0, 1]], base=0, channel_multiplier=1,
                           allow_small_or_imprecise_dtypes=True)
            row_f = r1.tile([128, 128], F32, tag="rowf")
            nc.gpsimd.iota(row_f[:], pattern=[[1, 128]], base=0, channel_multiplier=0,
                           allow_small_or_imprecise_dtypes=True)
            ident_f = pp.tile([128, 128], F32, tag="identf")
            nc.vector.tensor_scalar(ident_f[:], row_f[:], pidx_f[:], None, op0=ALU.is_equal)
            ident_b = pp.tile([128, 128], BF16, tag="identb")
            nc.vector.tensor_copy(ident_b[:], ident_f[:])

            # ---------------- gathers: per-node projection rows ----------------
            Gz = pp.tile([BL, L, H], BF16, tag="Gz")
            Gh = pp.tile([BL, L, H], BF16, tag="Gh")
            Gr = pp.tile([BL, L, H], BF16, tag="Gr")
            Gu = pp.tile([BL, L, H], BF16, tag="Gu")
            for l in range(L):
                for gt, dt_ in ((Gz, d_Pz), (Gh, d_Ph), (Gr, d_Pr), (Gu, d_Pu)):
                    nc.gpsimd.indirect_dma_start(
                        out=gt[:, l, :], out_offset=None, in_=dt_[:],
                        in_offset=bass.IndirectOffsetOnAxis(ap=wid_s[:, l:l + 1], axis=0),
                    )

            # ---------------- tree-vec projections ----------------
            # tvT = tv.T  (PE transpose, fp32)
            ps_tv = ps_rec.tile([128, 512], F32, tag="p")
            nc.tensor.transpose(ps_tv[:, :LAT], tv_s[:], ident_f[:])
            tvT = pp.tile([LAT, BL], F32, tag="tvT")
            nc.scalar.copy(tvT[:], ps_tv[:, :LAT])
            # Utv = tv @ Wutv  [BL, H] (fp32 matmul, one-shot)
            ps_utv = ps_rec.tile([128, 512], F32, tag="p")
            nc.tensor.matmul(ps_utv[:], tvT[:], w_Wutv[:], start=True, stop=True)
            utv_b = pp.tile([BL, H], BF16, tag="utv")
            nc.scalar.copy(utv_b[:], ps_utv[:])
            # Gu_c[l] = Gu[l] + Utv  (bf16)
            for l in range(L):
                nc.vector.tensor_add(Gu[:, l, :], Gu[:, l, :], utv_b[:])
            # WtvcT = (tv @ Wwtv + bw).T  [H-part, BL] bf16 (q first layer is
            # transposed; bias bw enters per-partition here)
            wtvcT = pp.tile([128, KH, BL], BF16, tag="wtvcT")
            for j in range(KH):
                ps_w = ps_rec.tile([128, 512], F32, tag="p")
                nc.tensor.matmul(ps_w[:, :BL], w_Wwtv[:, 128 * j:128 * (j + 1)], tvT[:],
                                 start=True, stop=True)
                nc.scalar.activation(wtvcT[:, j, :], ps_w[:, :BL], AF.Identity,
                                     bias=bw_s[:, j:j + 1])

            # ---------------- accumulators ----------------
            acc = pp.tile([BL, 4], F32, tag="acc")  # qloss, ploss, qacc, pacc
            nc.vector.memset(acc[:], 0.0)

            # fwd hsT persist (reused as m_fwd in bwd phase + q inputs)
            hsT_f = pp.tile([128, L - 1, KH, BL], BF16, tag="hsTf")

            # per-row loss stashes (losses batched at the end to avoid ACT
            # LUT-table thrash between sigmoid and exp/ln tables)
            pvals = pp.tile([BL, T + 1], F32, tag="pvals")
            q_S = pp.tile([BL, L], F32, tag="qS")
            q_mx = pp.tile([BL, L], F32, tag="qmx")
            q_tg = pp.tile([BL, L], F32, tag="qtg")

            # ---------------- helpers ----------------
            def emit_p(hT, v_node, row):
                """p-row: pre = Gu_c[v] + h @ Uh; stash p = relu(pre)@Us + bs."""
                psp = ps_rec.tile([BL, H], F32, tag="p")
                nc.tensor.matmul(psp[:], ident_b[:], Gu[:, v_node, :], start=True,
                                 stop=(hT is None))
                if hT is not None:
                    for k in range(KH):
                        nc.tensor.matmul(psp[:], hT[:, k, :], w_Uh[:, k, :],
                                         start=False, stop=(k == KH - 1))
                # p-layer columns are pre-scaled by |Us| and sign-permuted on
                # the host, so p = sum(relu[:npos]) - sum(relu[npos:]) + bs and
                # the ACT relu's accum_out gives both sums for free
                prelu = rgs.tile([BL, H], BF16, tag="prelu")
                accp = rg.tile([BL, 1], F32, tag="accp")
                accn = rg.tile([BL, 1], F32, tag="accn")
                if npos == 0:
                    nc.scalar.activation(prelu[:], psp[:], AF.Relu, accum_out=accn[:])
                    nc.vector.scalar_tensor_tensor(
                        pvals[:, row:row + 1], accn[:], -1.0, bs_b[:, 0:1],
                        op0=ALU.mult, op1=ALU.add)
                elif npos == H:
                    nc.scalar.activation(prelu[:], psp[:], AF.Relu, accum_out=accp[:])
                    nc.vector.tensor_add(pvals[:, row:row + 1], accp[:], bs_b[:, 0:1])
                else:
                    nc.scalar.activation(prelu[:, :npos], psp[:, :npos], AF.Relu,
                                         accum_out=accp[:])
                    nc.scalar.activation(prelu[:, npos:], psp[:, npos:], AF.Relu,
                                         accum_out=accn[:])
                    nc.vector.scalar_tensor_tensor(
                        pvals[:, row:row + 1], accp[:], bs_b[:, 0:1], accn[:],
                        op0=ALU.add, op1=ALU.subtract)

            def emit_q(hT, l_node):
                """q-row for node l: logits = relu(h@Wwh + tv@Wwtv + bw) @ Wo + bo.
                Stashes S=sum(exp(q)), max(q), q[tgt]; ln/compare happen later."""
                psqT = ps_rec.tile([128, KH, BL], F32, tag="z")
                for j in range(KH):
                    nc.tensor.matmul(psqT[:, j, :], ident_b[:], wtvcT[:, j, :],
                                     start=True, stop=(hT is None))
                    if hT is not None:
                        for k in range(KH):
                            nc.tensor.matmul(psqT[:, j, :],
                                             w_Wwh[:, k, 128 * j:128 * (j + 1)],
                                             hT[:, k, :], start=False,
                                             stop=(k == KH - 1))
                qreluT = rg.tile([128, KH, BL], BF16, tag="qreluT")
                nc.scalar.activation(qreluT[:], psqT[:], AF.Relu)
                psq1 = ps_rec.tile([BL, 512], F32, tag="h")
                psq2 = ps_rec.tile([BL, V - 512], F32, tag="r")
                nc.tensor.matmul(psq1[:], ident_b[:], bo_b[:, :512], start=True, stop=False)
                nc.tensor.matmul(psq2[:], ident_b[:], bo_b[:, 512:], start=True, stop=False)
                for k in range(KH):
                    nc.tensor.matmul(psq1[:], qreluT[:, k, :], w_Wo[:, k, :512],
                                     start=False, stop=(k == KH - 1))
                    nc.tensor.matmul(psq2[:], qreluT[:, k, :], w_Wo[:, k, 512:],
                                     start=False, stop=(k == KH - 1))
                m1 = rg.tile([BL, 1], F32, tag="m1")
                m2 = rg.tile([BL, 1], F32, tag="m2")
                nc.vector.tensor_reduce(m1[:], psq1[:], axis=mybir.AxisListType.X, op=ALU.max)
                nc.vector.tensor_reduce(m2[:], psq2[:], axis=mybir.AxisListType.X, op=ALU.max)
                nc.vector.tensor_max(q_mx[:, l_node:l_node + 1], m1[:], m2[:])
                # |q| is O(1) here, so exp without max-shift is safe in fp32
                e1 = rgs.tile([BL, 512], BF16, tag="e1")
                e2 = rgs.tile([BL, V - 512], BF16, tag="e2")
                s1 = rg.tile([BL, 1], F32, tag="s1")
                s2 = rg.tile([BL, 1], F32, tag="s2")
                nc.scalar.activation(e1[:], psq1[:], AF.Exp, accum_out=s1[:])
                nc.scalar.activation(e2[:], psq2[:], AF.Exp, accum_out=s2[:])
                nc.vector.tensor_add(q_S[:, l_node:l_node + 1], s1[:], s2[:])
                # q[tgt] via one-hot dot against the two psum halves
                mask = rgs.tile([BL, V], F32, tag="mask")
                nc.vector.tensor_scalar(mask[:], iota_f[:], tgtf[:, l_node:l_node + 1],
                                        None, op0=ALU.is_equal)
                scr1 = rgs.tile([BL, 512], F32, tag="scr1")
                scr2 = rgs.tile([BL, V - 512], F32, tag="scr2")
                qt1 = rg.tile([BL, 1], F32, tag="qt1")
                qt2 = rg.tile([BL, 1], F32, tag="qt2")
                nc.vector.tensor_mul(scr1[:], psq1[:], mask[:, :512])
                nc.vector.tensor_mul(scr2[:], psq2[:], mask[:, 512:])
                nc.vector.tensor_reduce(qt1[:], scr1[:], axis=mybir.AxisListType.X, op=ALU.add)
                nc.vector.tensor_reduce(qt2[:], scr2[:], axis=mybir.AxisListType.X, op=ALU.add)
                nc.vector.tensor_add(q_tg[:, l_node:l_node + 1], qt1[:], qt2[:])

            # ---------------- root p-row ----------------
            emit_p(None, 0, 0)

            # q rows are deferred into batches so their exp-table ACT ops
            # don't interleave with the recurrence's sigmoid-table ops
            q_pending = [(None, 0)]

            # ---------------- 46 DFS steps ----------------
            sT = None        # bf16 [128, KH, BL] transposed prev m_new
            armT = None      # bf16 transposed prev rm
            s_nat = None     # f32 [BL, H] prev m_new (natural)
            for t in range(T):
                u, v = int(SRC[t]), int(DST[t])
                fwd = t < L - 1
                fresh = t == 0 or t == L - 1  # s = arm = 0 at these steps

                # --- h then z pre-activations (tanh is needed first) ---
                psh = ps_rec.tile([BL, H], F32, tag="h")
                nc.tensor.matmul(psh[:], ident_b[:], Gh[:, u, :], start=True,
                                 stop=fresh)
                psz = ps_rec.tile([BL, H], F32, tag="z")
                if not fresh:
                    for k in range(KH):
                        nc.tensor.matmul(psh[:], armT[:, k, :], w_Whm[:, k, :],
                                         start=False, stop=(k == KH - 1))
                nc.tensor.matmul(psz[:], ident_b[:], Gz[:, u, :], start=True,
                                 stop=fresh)
                if not fresh:
                    for k in range(KH):
                        nc.tensor.matmul(psz[:], sT[:, k, :], w_Wzs[:, k, :],
                                         start=False, stop=(k == KH - 1))
                # --- m_new = s + z*(2g - 1 - s), pipelined per H-half ---
                z_nat = rg.tile([BL, H], F32, tag="znat")
                g_nat = rg.tile([BL, H], F32, tag="tnat")
                m_nat = rg.tile([BL, H], F32, tag="mnat")
                m_bf = rg.tile([BL, H], BF16, tag="mbf")
                if fwd:
                    mT = hsT_f[:, t]  # fwd h == m_new: write straight into hsT
                else:
                    mT = rg.tile([128, KH, BL], BF16, tag="mT")
                HF = H // 2
                for hh in range(2):
                    sl = slice(hh * HF, (hh + 1) * HF)
                    # tanh(x) = 2*sigmoid(2x) - 1 keeps ACT in the sigmoid table
                    nc.scalar.activation(g_nat[:, sl], psh[:, sl], AF.Sigmoid, scale=2.0)
                    nc.scalar.activation(z_nat[:, sl], psz[:, sl], AF.Sigmoid)
                    if fresh:
                        nc.vector.tensor_scalar(g_nat[:, sl], g_nat[:, sl], 2.0, -1.0,
                                                op0=ALU.mult, op1=ALU.add)
                        nc.vector.tensor_mul(m_bf[:, sl], z_nat[:, sl], g_nat[:, sl])
                        nc.vector.tensor_mul(m_nat[:, sl], z_nat[:, sl], g_nat[:, sl])
                    else:
                        nc.vector.scalar_tensor_tensor(
                            g_nat[:, sl], g_nat[:, sl], 2.0, s_nat[:, sl],
                            op0=ALU.mult, op1=ALU.subtract)
                        nc.vector.scalar_tensor_tensor(
                            g_nat[:, sl], g_nat[:, sl], 1.0, z_nat[:, sl],
                            op0=ALU.subtract, op1=ALU.mult)
                        nc.vector.tensor_add(m_bf[:, sl], s_nat[:, sl], g_nat[:, sl])
                        nc.vector.tensor_add(m_nat[:, sl], s_nat[:, sl], g_nat[:, sl])
                    pst_m = ps_trm.tile([128, 2, BL], BF16, tag="trm")
                    for j in range(2):
                        k = 2 * hh + j
                        nc.tensor.transpose(pst_m[:, j, :],
                                            m_bf[:, 128 * k:128 * (k + 1)], ident_b[:])
                    nc.scalar.copy(mT[:, 2 * hh:2 * hh + 2, :], pst_m[:])

                # --- r = sigmoid(Gr[v] + m_new @ Ur) ; rm = r*m_new ---
                psr = ps_rec.tile([BL, H], F32, tag="r")
                nc.tensor.matmul(psr[:], ident_b[:], Gr[:, v, :], start=True, stop=False)
                for k in range(KH):
                    nc.tensor.matmul(psr[:], mT[:, k, :], w_Ur[:, k, :],
                                     start=False, stop=(k == KH - 1))
                r_nat = rg.tile([BL, H], F32, tag="rnat")
                rm_bf = rg.tile([BL, H], BF16, tag="rmbf")
                rmT = rg.tile([128, KH, BL], BF16, tag="rmT")
                for hh in range(2):
                    sl = slice(hh * HF, (hh + 1) * HF)
                    nc.scalar.activation(r_nat[:, sl], psr[:, sl], AF.Sigmoid)
                    nc.vector.tensor_mul(rm_bf[:, sl], r_nat[:, sl], m_nat[:, sl])
                    pst_r = ps_trr.tile([128, 2, BL], BF16, tag="trr")
                    for j in range(2):
                        k = 2 * hh + j
                        nc.tensor.transpose(pst_r[:, j, :],
                                            rm_bf[:, 128 * k:128 * (k + 1)], ident_b[:])
                    nc.vector.tensor_copy(rmT[:, 2 * hh:2 * hh + 2, :], pst_r[:])

                # --- h_v (transposed): fwd = mT (already in hsT); bwd = mT + m_fwd ---
                if fwd:
                    hT = hsT_f[:, t]
                else:
                    if t < T - 1:
                        hT = rg.tile([128, KH, BL], BF16, tag="hTb")
                        nc.vector.tensor_add(hT[:], mT[:], hsT_f[:, 44 - t])
                    else:
                        hT = mT  # v == 0: no incoming fwd edge

                # --- p row for this step; q rows batched ---
                emit_p(hT, v, t + 1)
                if fwd:
                    q_pending.append((hsT_f[:, t], t + 1))

                sT, armT, s_nat = mT, rmT, m_nat

            # ---------------- deferred q rows (one ACT-table regime) ----------------
            # scheduler fence: keep the exp-table phase strictly after the
            # sigmoid-table loop so ACT LUT loads don't thrash
            tc.no_sync_barrier()
            if with_q:
                for qhT, ql in q_pending:
                    emit_q(qhT, ql)

            # ---------------- batched losses ----------------
            # p: ploss_row = ln(1+exp(p)) - p*tgt ; pacc_row = ((p>0) == tgt)
            pm = pp.tile([BL, T + 1], F32, tag="pm")
            nc.vector.memset(pm[:, :L - 1], 1.0)
            nc.vector.memset(pm[:, L - 1:], 0.0)
            pe = rgs.tile([BL, T + 1], F32, tag="pe")
            nc.scalar.activation(pe[:], pvals[:], AF.Exp)
            nc.vector.tensor_scalar_add(pe[:], pe[:], 1.0)
            sp = rgs.tile([BL, T + 1], F32, tag="spl")
            nc.scalar.activation(sp[:], pe[:], AF.Ln)
            ptd = rgs.tile([BL, T + 1], F32, tag="ptd")
            nc.vector.tensor_mul(ptd[:], pvals[:], pm[:])
            nc.vector.tensor_sub(ptd[:], sp[:], ptd[:])
            acc = pp.tile([BL, 4], F32, tag="acc")  # qloss, ploss, qacc, pacc
            nc.vector.tensor_reduce(acc[:, 1:2], ptd[:], axis=mybir.AxisListType.X,
                                    op=ALU.add)
            pg = rgs.tile([BL, T + 1], F32, tag="pg")
            nc.vector.tensor_scalar(pg[:], pvals[:], 0.0, None, op0=ALU.is_gt)
            nc.vector.tensor_tensor(pg[:], pg[:], pm[:], ALU.is_equal)
            nc.vector.tensor_reduce(acc[:, 3:4], pg[:], axis=mybir.AxisListType.X,
                                    op=ALU.add)
            # q: qloss_row = ln(S) - q[tgt] ; qacc_row = (q[tgt] == max)
            if not with_q:
                nc.vector.memset(q_S[:], 1.0)
                nc.vector.memset(q_mx[:], 0.0)
                nc.vector.memset(q_tg[:], 0.0)
            lns = rgs.tile([BL, L], F32, tag="lns")
            nc.scalar.activation(lns[:], q_S[:], AF.Ln)
            nc.vector.tensor_sub(lns[:], lns[:], q_tg[:])
            nc.vector.tensor_reduce(acc[:, 0:1], lns[:], axis=mybir.AxisListType.X,
                                    op=ALU.add)
            qe = rgs.tile([BL, L], F32, tag="qe")
            nc.vector.tensor_tensor(qe[:], q_tg[:], q_mx[:], ALU.is_equal)
            nc.vector.tensor_reduce(acc[:, 2:3], qe[:], axis=mybir.AxisListType.X,
                                    op=ALU.add)

            # ---------------- final cross-partition reduce ----------------
            ones = pp.tile([128, 1], F32, tag="ones")
            nc.vector.memset(ones[:], 1.0)
            psf = ps_rec.tile([128, 512], F32, tag="p")
            nc.tensor.matmul(psf[:1, :4], ones[:], acc[:], start=True, stop=True)
            outs = pp.tile([1, 4], F32, tag="outs")
            nc.scalar.copy(outs[:], psf[:1, :4])
            nc.sync.dma_start(d_out[:], outs[:])

    nc.compile()
    return nc


_CACHE = {}


def _prep_inputs(wid, tree_vec, emb, Wz, bz, Wr, Ur, bur, Wh, bh, Ww, bw, Wu, bu,
                 Wo, bo, Us, bs):
    emb64 = emb.astype(np.float64)
    Pz = (emb64 @ Wz[:H].astype(np.float64) + bz).astype(_BF)
    Ph = (emb64 @ Wh[:H].astype(np.float64) + bh).astype(_BF)
    Pr = (emb64 @ Wr.astype(np.float64) + bur).astype(_BF)
    us = np.asarray(Us)[:, 0].astype(np.float64)
    perm = np.concatenate([np.nonzero(us >= 0)[0], np.nonzero(us < 0)[0]])
    ausp = np.abs(us)[None, perm]
    Pu = (((emb64 @ Wu[:H].astype(np.float64) + bu)[:, perm]) * ausp).astype(_BF)

    def wlay(w):  # [K, N] -> [128, K/128, N]
        return np.ascontiguousarray(
            w.reshape(w.shape[0] // 128, 128, w.shape[1]).transpose(1, 0, 2)
        ).astype(_BF)

    shared = dict(
        Pz=Pz, Ph=Ph, Pr=Pr, Pu=Pu,
        Wzs=wlay(Wz[H:]), Whm=wlay(Wh[H:]), Ur=wlay(Ur),
        Uh=wlay((Wu[H:2 * H].astype(np.float64)[:, perm] * ausp)),
        Wwh=wlay(Ww[:H]), Wo=wlay(Wo),
        Wutv=np.ascontiguousarray(Wu[2 * H:].astype(np.float64)[:, perm] * ausp).astype(np.float32),
        Wwtv=np.ascontiguousarray(Ww[H:]).astype(np.float32),
        bwq=np.ascontiguousarray(bw.reshape(KH, 128).T).astype(np.float32),
        bs=bs.reshape(1, 1).astype(np.float32),
        bo=bo[None, :].astype(np.float32),
    )
    in_maps = []
    for c in range(NCORES):
        sl = slice(c * BL, (c + 1) * BL)
        m = dict(shared)
        m["wid"] = np.ascontiguousarray(wid[sl]).astype(np.int32)
        m["tv"] = np.ascontiguousarray(tree_vec[sl]).astype(np.float32)
        in_maps.append(m)
    return in_maps


def kernel(**inputs):
    npa = {k: np.asarray(v) for k, v in inputs.items()}
    npos = int((npa["Us"][:, 0] >= 0).sum())
    if _CACHE.get("npos") != npos:
        _CACHE["nc"] = _build_nc(npos)
        _CACHE["npos"] = npos
    nc = _CACHE["nc"]
    in_maps = _prep_inputs(**npa)
    res = run_bass_kernel_spmd(nc, in_maps, core_ids=list(range(NCORES)))
    _CACHE["last_res"] = res
    tot = np.zeros(4, dtype=np.float64)
    for r in res.results:
        tot += r["out"][0].astype(np.float64)
    q_loss = tot[0] / B
    p_loss = tot[1] / B
    q_acc = tot[2] / (L * B)
    p_acc = tot[3] / ((T + 1) * B)
    return np.array([q_loss, p_loss, q_acc, p_acc], dtype=np.float32)



# revision 19
# speedup vs baseline: 1.1591x; 1.1591x over previous
"""DGLJTNN decoder on 8 Trainium2 NeuronCores.

Data-parallel over the tree batch dim B=1024: each core handles 128 trees.
Chain-tree DFS schedule is static; the per-edge GRU recurrence runs 46
sequential steps on-device.

Key design (v2 rewrite):
- Word-embedding projections are host-folded into ONE per-vocab table
  [V, 4H] (z|h|r|u blocks) and gathered with node-chunked indirect DMAs
  (6 SWDGE instructions instead of 96).
- The recurrence uses a tanh-only formulation (sigmoid(x) = (1+tanh(x/2))/2
  with the /2 folded into host tables), so every ACT op in the kernel body
  (tanh, relu, exp, identity) lives in ONE activation table set and the
  per-step q-row exp ops interleave with the recurrence at zero table-load
  cost.  ln runs once at the end after a single table switch.
- GRU state is bf16 end-to-end; DVE elementwise ops run in 2x perf mode.
- p-rows and q-rows are emitted INSIDE the 46-step loop so their work hides
  in the recurrence's idle engine slots (the loop is latency-bound).
  p-row tail: Pool relu + DVE fused mult-reduce against a signed Us
  broadcast.  q-row: natural-domain hidden + PE transpose + Wo logits,
  exp+accum straight from PSUM, target/max extraction on Pool+DVE.
"""

import os
import numpy as np
import ml_dtypes

import concourse.bass as bass
import concourse.mybir as mybir
import concourse.tile as tile
from concourse import bacc
from concourse.bass_utils import run_bass_kernel_spmd

F32 = mybir.dt.float32
BF16 = mybir.dt.bfloat16
I32 = mybir.dt.int32
AF = mybir.ActivationFunctionType
ALU = mybir.AluOpType

B, L, H, LAT, V = 1024, 24, 512, 128, 800
E = 2 * (L - 1)
T = E
NCORES = 8
BL = B // NCORES  # 128 trees per core
KH = H // 128     # 4 K-tiles of 128 over H
HF = H // 2

# static DFS schedule (chain tree), mirrors reference.py
_f = np.arange(L - 1)
_b = np.arange(L - 2, -1, -1)
SRC = np.concatenate([_f, _b + 1])
DST = np.concatenate([_f + 1, _b])

_BF = ml_dtypes.bfloat16


DBG_NO_Q = bool(int(os.environ.get("DBG_NO_Q", "0")))
DBG_Q_STAGE = int(os.environ.get("DBG_Q_STAGE", "3"))
DBG_P_STAGE = int(os.environ.get("DBG_P_STAGE", "2"))
DBG_NO_P = bool(int(os.environ.get("DBG_NO_P", "0")))


def _build_nc():
    nc = bacc.Bacc("TRN2", target_bir_lowering=False, debug=False, num_devices=NCORES)

    # ---------------- DRAM I/O ----------------
    d_wid = nc.dram_tensor("wid", [BL, L], I32, kind="ExternalInput")
    d_tv = nc.dram_tensor("tv", [BL, LAT], F32, kind="ExternalInput")
    # merged per-vocab projection table: [V, 4, H] blocks z|h|r|u
    # z block = (emb@Wz1 + bz)/2 ; h = emb@Wh1 + bh ; r = (emb@Wr + bur)/2 ;
    # u = emb@Wu[:H] + bu
    d_P = nc.dram_tensor("P", [V, 4 * H], BF16, kind="ExternalInput")
    # recurrent / late weights, bf16, layout [128, K/128, N]
    d_Wzs = nc.dram_tensor("Wzs", [128, KH, H], BF16, kind="ExternalInput")  # /2
    d_Whm = nc.dram_tensor("Whm", [128, KH, H], BF16, kind="ExternalInput")  # /2
    d_Ur = nc.dram_tensor("Ur", [128, KH, H], BF16, kind="ExternalInput")    # /4
    d_Uh = nc.dram_tensor("Uh", [128, KH, H], BF16, kind="ExternalInput")
    d_Wwh = nc.dram_tensor("Wwh", [128, KH, H], BF16, kind="ExternalInput")
    d_Wo = nc.dram_tensor("Wo", [128, KH, V], BF16, kind="ExternalInput")
    d_Wutv = nc.dram_tensor("Wutv", [LAT, H], F32, kind="ExternalInput")
    d_Wwtv = nc.dram_tensor("Wwtv", [LAT, H], F32, kind="ExternalInput")
    d_bw = nc.dram_tensor("bw", [1, H], F32, kind="ExternalInput")
    d_us = nc.dram_tensor("us", [1, H], F32, kind="ExternalInput")  # signed Us col
    d_bs = nc.dram_tensor("bs", [1, 1], F32, kind="ExternalInput")
    d_bo = nc.dram_tensor("bo", [1, V], F32, kind="ExternalInput")
    d_out = nc.dram_tensor("out", [1, 4], F32, kind="ExternalOutput")

    with tile.TileContext(nc) as tc:
        with (
            tc.tile_pool(name="persist", bufs=1) as pp,
            tc.tile_pool(name="ring", bufs=2) as rg,
            tc.tile_pool(name="rgs", bufs=1) as rgs,
            tc.tile_pool(name="ps_z", bufs=1, space="PSUM") as ps_z,
            tc.tile_pool(name="ps_h", bufs=1, space="PSUM") as ps_h,
            tc.tile_pool(name="ps_r", bufs=1, space="PSUM") as ps_r,
            tc.tile_pool(name="ps_t", bufs=1, space="PSUM") as ps_t,
            tc.tile_pool(name="ps_pq", bufs=2, space="PSUM") as ps_pq,
            tc.tile_pool(name="ps_log", bufs=1, space="PSUM") as ps_log,
        ):
            # ---------------- setup ----------------
            wid_s = pp.tile([BL, L], I32, tag="wid")
            nc.sync.dma_start(wid_s[:], d_wid[:])
            tv_s = pp.tile([BL, LAT], F32, tag="tv")
            nc.sync.dma_start(tv_s[:], d_tv[:])

            # weights
            w_Wzs = pp.tile([128, KH, H], BF16, tag="Wzs")
            w_Whm = pp.tile([128, KH, H], BF16, tag="Whm")
            w_Ur = pp.tile([128, KH, H], BF16, tag="Ur")
            w_Uh = pp.tile([128, KH, H], BF16, tag="Uh")
            w_Wwh = pp.tile([128, KH, H], BF16, tag="Wwh")
            w_Wo = pp.tile([128, KH, V], BF16, tag="Wo")
            for dst, src in [(w_Wzs, d_Wzs), (w_Whm, d_Whm), (w_Ur, d_Ur),
                             (w_Uh, d_Uh), (w_Wwh, d_Wwh), (w_Wo, d_Wo)]:
                nc.sync.dma_start(dst[:], src[:])
            w_Wutv = pp.tile([LAT, H], F32, tag="Wutv")
            nc.sync.dma_start(w_Wutv[:], d_Wutv[:])
            w_Wwtv = pp.tile([LAT, H], F32, tag="Wwtv")
            nc.sync.dma_start(w_Wwtv[:], d_Wwtv[:])
            # broadcast rows
            bw_b = rgs.tile([BL, H], F32, tag="bwb")
            nc.sync.dma_start(bw_b[:], bass.AP(tensor=d_bw, offset=0, ap=[[0, BL], [1, H]]))
            us_b = pp.tile([BL, H], BF16, tag="usb")
            us_f = rgs.tile([BL, H], F32, tag="usf")
            nc.sync.dma_start(us_f[:], bass.AP(tensor=d_us, offset=0, ap=[[0, BL], [1, H]]))
            nc.vector.tensor_copy(us_b[:], us_f[:])
            bs_b = pp.tile([BL, 1], F32, tag="bs")
            nc.sync.dma_start(bs_b[:], bass.AP(tensor=d_bs, offset=0, ap=[[0, BL], [1, 1]]))
            bo_f = rgs.tile([BL, V], F32, tag="bof")
            nc.sync.dma_start(bo_f[:], bass.AP(tensor=d_bo, offset=0, ap=[[0, BL], [1, V]]))
            bo_b = pp.tile([BL, V], BF16, tag="bo")
            nc.scalar.copy(bo_b[:], bo_f[:])

            # iota row 0..V-1 (fp32, exact) + fp32 targets
            iota_f = pp.tile([BL, V], F32, tag="iota")
            nc.gpsimd.iota(iota_f[:], pattern=[[1, V]], base=0, channel_multiplier=0,
                           allow_small_or_imprecise_dtypes=True)
            tgtf = pp.tile([BL, L], F32, tag="tgtf")
            nc.vector.tensor_copy(tgtf[:], wid_s[:])

            # identities (fp32 + bf16) via iota compare
            pidx_f = rgs.tile([128, 1], F32, tag="pidxf")
            nc.gpsimd.iota(pidx_f[:], pattern=[[0, 1]], base=0, channel_multiplier=1,
                           allow_small_or_imprecise_dtypes=True)
            row_f = rgs.tile([128, 128], F32, tag="rowf")
            nc.gpsimd.iota(row_f[:], pattern=[[1, 128]], base=0, channel_multiplier=0,
                           allow_small_or_imprecise_dtypes=True)
            ident_f = pp.tile([128, 128], F32, tag="identf")
            nc.vector.tensor_scalar(ident_f[:], row_f[:], pidx_f[:], None, op0=ALU.is_equal)
            ident_b = pp.tile([128, 128], BF16, tag="identb")
            nc.vector.tensor_copy(ident_b[:], ident_f[:])

            # ---------------- tree-vec projections ----------------
            # tvT = tv.T  (PE transpose, fp32)
            ps_tv = ps_log.tile([BL, 512], F32, tag="log1")
            nc.tensor.transpose(ps_tv[:, :LAT], tv_s[:], ident_f[:])
            tvT = rgs.tile([LAT, BL], F32, tag="tvT")
            nc.scalar.copy(tvT[:], ps_tv[:, :LAT])
            # utv = tv @ Wutv  [BL, H]  (p first layer tv-part)
            ps_utv = ps_log.tile([BL, 512], F32, tag="log1")
            nc.tensor.matmul(ps_utv[:], tvT[:], w_Wutv[:], start=True, stop=True)
            utv_b = pp.tile([BL, H], BF16, tag="utv")
            nc.scalar.copy(utv_b[:], ps_utv[:])
            # wtvc = tv @ Wwtv + bw  [BL, H]  (q first layer tv-part)
            ps_wtv = ps_log.tile([BL, 512], F32, tag="log1")
            nc.tensor.matmul(ps_wtv[:], tvT[:], w_Wwtv[:], start=True, stop=True)
            wtvc_f = rgs.tile([BL, H], F32, tag="wtvf")
            nc.vector.tensor_add(wtvc_f[:], ps_wtv[:], bw_b[:])
            wtvc = pp.tile([BL, H], BF16, tag="wtvc")
            nc.vector.tensor_copy(wtvc[:], wtvc_f[:])

            # ---------------- gathers: merged table, node-chunked ----------------
            # Gall[b, l, g, :] = P[wid[b,l], g*H:(g+1)*H], g in {z,h,r,u}
            Gall = pp.tile([BL, L, 4 * H], BF16, tag="Gall")
            for l in range(L):
                nc.gpsimd.indirect_dma_start(
                    out=Gall[:, l, :], out_offset=None, in_=d_P[:],
                    in_offset=bass.IndirectOffsetOnAxis(ap=wid_s[:, l:l + 1], axis=0),
                )
                # fold utv into the u-block of this node (p-row input)
                nc.vector.tensor_add(Gall[:, l, 3 * H:], Gall[:, l, 3 * H:], utv_b[:])

            # ---------------- accumulators / stashes ----------------
            hsT_f = pp.tile([128, L - 1, KH, BL], BF16, tag="hsTf")  # fwd h (=m) transposed
            pvals = pp.tile([BL, T + 1], F32, tag="pvals")
            q_S = pp.tile([BL, L], F32, tag="qS")
            q_mx = pp.tile([BL, L], F32, tag="qmx")
            q_tg = pp.tile([BL, L], F32, tag="qtg")

            # ---------------- helpers ----------------
            def emit_p_mm(hT, v_node):
                """p-row pre-activation: psp = Gu_c[v] + h @ Uh  [BL, H]."""
                psp = ps_pq.tile([BL, H], F32, tag="pq")
                nc.tensor.matmul(psp[:], ident_b[:], Gall[:, v_node, 3 * H:4 * H],
                                 start=True, stop=(hT is None))
                if hT is not None:
                    for k in range(KH):
                        nc.tensor.matmul(psp[:], hT[:, k, :], w_Uh[:, k, :],
                                         start=False, stop=(k == KH - 1))
                return psp

            def emit_p_post(psp, row):
                """p-row tail: p = relu(psp) . us  (signed; bs added at the end).
                DVE relu (Pool cannot read PSUM), then DVE fused mult+reduce."""
                prelu = rg.tile([BL, H], BF16, tag="prelu")
                nc.scalar.activation(prelu[:], psp[:], AF.Relu)
                pscr = rg.tile([BL, H], F32, tag="pscr")
                nc.vector.tensor_mul(pscr[:], prelu[:], us_b[:])
                nc.vector.tensor_reduce(pvals[:, row:row + 1], pscr[:],
                                        axis=mybir.AxisListType.X, op=ALU.add)

            def emit_q_mm(hT, l_node):
                """q-row hidden pre-activation [BL, H] (natural domain)."""
                psq = ps_pq.tile([BL, H], F32, tag="pq")
                nc.tensor.matmul(psq[:], ident_b[:], wtvc[:], start=True,
                                 stop=(hT is None))
                if hT is not None:
                    for k in range(KH):
                        nc.tensor.matmul(psq[:], hT[:, k, :], w_Wwh[:, k, :],
                                         start=False, stop=(k == KH - 1))
                return psq

            def emit_q_mid(psq):
                """relu -> transpose -> qreluT sbuf; then logits psum [BL, V]."""
                qrelu = rg.tile([BL, H], BF16, tag="qrelu")
                nc.scalar.activation(qrelu[:], psq[:], AF.Relu)
                pst_q = ps_t.tile([128, KH, BL], BF16, tag="tr")
                for k in range(KH):
                    nc.tensor.transpose(pst_q[:, k, :],
                                        qrelu[:, 128 * k:128 * (k + 1)], ident_b[:])
                qreluT = rg.tile([128, KH, BL], BF16, tag="qreluT")
                nc.vector.tensor_copy(qreluT[:], pst_q[:])
                log1 = ps_log.tile([BL, 512], F32, tag="log1")
                log2 = ps_log.tile([BL, V - 512], F32, tag="log2")
                nc.tensor.matmul(log1[:], ident_b[:], bo_b[:, :512],
                                 start=True, stop=False)
                nc.tensor.matmul(log2[:], ident_b[:], bo_b[:, 512:],
                                 start=True, stop=False)
                for k in range(KH):
                    nc.tensor.matmul(log1[:], qreluT[:, k, :], w_Wo[:, k, :512],
                                     start=False, stop=(k == KH - 1))
                    nc.tensor.matmul(log2[:], qreluT[:, k, :], w_Wo[:, k, 512:],
                                     start=False, stop=(k == KH - 1))
                return (log1, log2)

            def emit_q_post(pslog, l_node):
                """S = sum(exp(q)) via ACT accum; max via DVE; q[tgt] via
                mask + DVE fused mult-reduce.  Two psum tiles (bank-sized)."""
                log1, log2 = pslog
                qexp = rgs.tile([BL, V], BF16, tag="qexp")
                s1 = rg.tile([BL, 1], F32, tag="s1")
                s2 = rg.tile([BL, 1], F32, tag="s2")
                nc.scalar.activation(qexp[:, :512], log1[:], AF.Exp, accum_out=s1[:])
                nc.scalar.activation(qexp[:, 512:], log2[:], AF.Exp, accum_out=s2[:])
                nc.vector.tensor_add(q_S[:, l_node:l_node + 1], s1[:], s2[:])
                m1 = rg.tile([BL, 1], F32, tag="m1")
                m2 = rg.tile([BL, 1], F32, tag="m2")
                nc.vector.tensor_reduce(m1[:], log1[:],
                                        axis=mybir.AxisListType.X, op=ALU.max)
                nc.vector.tensor_reduce(m2[:], log2[:],
                                        axis=mybir.AxisListType.X, op=ALU.max)
                nc.vector.tensor_max(q_mx[:, l_node:l_node + 1], m1[:], m2[:])
                mask = rgs.tile([BL, V], F32, tag="mask")
                nc.vector.tensor_scalar(mask[:], iota_f[:], tgtf[:, l_node:l_node + 1],
                                        None, op0=ALU.is_equal)
                mscr = rgs.tile([BL, V], F32, tag="mscr")
                t1 = rg.tile([BL, 1], F32, tag="t1")
                t2 = rg.tile([BL, 1], F32, tag="t2")
                nc.vector.tensor_mul(mscr[:, :512], log1[:], mask[:, :512])
                nc.vector.tensor_mul(mscr[:, 512:], log2[:], mask[:, 512:])
                nc.vector.tensor_reduce(t1[:], mscr[:, :512],
                                        axis=mybir.AxisListType.X, op=ALU.add)
                nc.vector.tensor_reduce(t2[:], mscr[:, 512:],
                                        axis=mybir.AxisListType.X, op=ALU.add)
                nc.vector.tensor_add(q_tg[:, l_node:l_node + 1], t1[:], t2[:])

            # ---------------- schedules for interleaved tail work ----------------
            # q-row k (k=0 root, k>=1 -> fwd step k-1) emitted during step sched_q[k]
            # (mm at that step, mid one step later, post one after).
            qrow_at_step = {}
            for k in range(L):
                s = max(1, min(T - 1, int(round(k * (T - 3) / (L - 1))) + 1))
                if k >= 1:
                    s = max(s, k + 1)  # h for fwd step k-1 ready after step k-1
                qrow_at_step.setdefault(s, []).append(k)

            # rotating state for deferred tail pieces
            pending_q = []   # (stage, data...) pushed through mm -> mid -> post
            pending_p = []   # (psp, row)

            # ---------------- GRU recurrence: 46 DFS steps ----------------
            # state (transposed, bf16): mT = m(t-1), rmTp = m + w*m (t-1)
            mT_prev = None
            rmT_prev = None
            pszh = None   # [BL, 2H] psum: cols 0:H = z-pre, H:2H = h-pre
            psr = None
            # prefetch inits for step 0
            psz = ps_z.tile([BL, H], F32, tag="z")
            psh = ps_h.tile([BL, H], F32, tag="h")
            u0 = int(SRC[0])
            nc.tensor.matmul(psz[:], ident_b[:], Gall[:, u0, 0:H], start=True, stop=True)
            nc.tensor.matmul(psh[:], ident_b[:], Gall[:, u0, H:2 * H], start=True, stop=True)
            psr = ps_r.tile([BL, H], F32, tag="r")
            v0 = int(DST[0])
            nc.tensor.matmul(psr[:], ident_b[:], Gall[:, v0, 2 * H:3 * H], start=True, stop=False)

            hT_last = None  # transposed h of previous step (for p-row t-1)

            for t in range(T):
                u, v = int(SRC[t]), int(DST[t])
                fwd = t < L - 1
                fresh = t == 0 or t == L - 1  # s = arm = 0 at these steps

                # --- accumulate z/h pre-activations onto prefetched inits ---
                if not fresh:
                    for k in range(KH):
                        nc.tensor.matmul(psz[:], mT_prev[:, k, :], w_Wzs[:, k, :],
                                         start=False, stop=(k == KH - 1))
                    for k in range(KH):
                        nc.tensor.matmul(psh[:], rmT_prev[:, k, :], w_Whm[:, k, :],
                                         start=False, stop=(k == KH - 1))

                # --- u_z = tanh(z-pre), g = tanh(h-pre); m = s + (1+u)/2*(g-s) ---
                # ACT emits both halves of both gates first (no head-of-line
                # blocking on the DVE->PE->copy pipeline), then per-half
                # DVE chain + transpose + copy.
                uz = rg.tile([BL, H], BF16, tag="uz")
                g_b = rg.tile([BL, H], BF16, tag="gb")
                m_bf = rg.tile([BL, H], BF16, tag="mbf")
                d_b = rg.tile([BL, H], BF16, tag="db")
                if fwd:
                    mT = hsT_f[:, t]
                else:
                    mT = rg.tile([128, KH, BL], BF16, tag="mT")
                for hh in range(2):
                    sl = slice(hh * HF, (hh + 1) * HF)
                    nc.scalar.activation(g_b[:, sl], psh[:, sl], AF.Tanh)
                    nc.scalar.activation(uz[:, sl], psz[:, sl], AF.Tanh)
                pst_m = ps_t.tile([128, KH, BL], BF16, tag="tr")
                for hh in range(2):
                    sl = slice(hh * HF, (hh + 1) * HF)
                    if fresh:
                        # m = (1+u)/2 * g = 0.5*(g + u*g)
                        nc.vector.tensor_mul(d_b[:, sl], uz[:, sl], g_b[:, sl])
                        nc.vector.tensor_add(d_b[:, sl], d_b[:, sl], g_b[:, sl])
                        nc.vector.tensor_scalar(m_bf[:, sl], d_b[:, sl], 0.5, None,
                                                op0=ALU.mult)
                    else:
                        # d = g - s ; a = u*d ; b = d + a ; m = s + 0.5*b
                        nc.vector.tensor_sub(d_b[:, sl], g_b[:, sl], s_bf[:, sl])
                        nc.vector.tensor_mul(g_b[:, sl], uz[:, sl], d_b[:, sl])
                        nc.vector.tensor_add(d_b[:, sl], d_b[:, sl], g_b[:, sl])
                        nc.vector.tensor_scalar(d_b[:, sl], d_b[:, sl], 0.5, None,
                                                op0=ALU.mult)
                        nc.vector.tensor_add(m_bf[:, sl], s_bf[:, sl], d_b[:, sl])
                    # transpose this half of m into mT (psum then copy)
                    for j in range(2):
                        k = 2 * hh + j
                        nc.tensor.transpose(pst_m[:, k, :],
                                            m_bf[:, 128 * k:128 * (k + 1)], ident_b[:])
                    nc.scalar.copy(mT[:, 2 * hh:2 * hh + 2, :], pst_m[:, 2 * hh:2 * hh + 2, :])
                    # psr k-tiles chase the mT copies (skipped when r unused)
                    if not (t == L - 2 or t == T - 1):
                        for j in range(2):
                            k = 2 * hh + j
                            nc.tensor.matmul(psr[:], mT[:, k, :], w_Ur[:, k, :],
                                             start=False, stop=(k == KH - 1))

                # --- w = tanh(r-pre); rm' = m + w*m (Whm pre-halved) ---
                psr_cur = psr
                last_rm = t == L - 2 or t == T - 1  # rm' unused after these steps
                if not last_rm:
                    w_b = rg.tile([BL, H], BF16, tag="wb")
                    rm_b = rg.tile([BL, H], BF16, tag="rmb")
                    rmT = rg.tile([128, KH, BL], BF16, tag="rmT")
                    pst_r = ps_t.tile([128, KH, BL], BF16, tag="tr")
                    for hh in range(2):
                        sl = slice(hh * HF, (hh + 1) * HF)
                        nc.scalar.activation(w_b[:, sl], psr_cur[:, sl], AF.Tanh)
                    for hh in range(2):
                        sl = slice(hh * HF, (hh + 1) * HF)
                        nc.vector.tensor_mul(rm_b[:, sl], w_b[:, sl], m_bf[:, sl])
                        nc.vector.tensor_add(rm_b[:, sl], rm_b[:, sl], m_bf[:, sl])
                        for j in range(2):
                            k = 2 * hh + j
                            nc.tensor.transpose(pst_r[:, k, :],
                                                rm_b[:, 128 * k:128 * (k + 1)], ident_b[:])
                        nc.vector.tensor_copy(rmT[:, 2 * hh:2 * hh + 2, :], pst_r[:, 2 * hh:2 * hh + 2, :])
                else:
                    rmT = None

                # --- h_v transposed: fwd = mT; bwd = mT + m_fwd[v] ---
                if fwd:
                    hT = hsT_f[:, t]
                else:
                    if t < T - 1:
                        hT = rg.tile([128, KH, BL], BF16, tag="hTb")
                        nc.vector.tensor_add(hT[:], mT[:], hsT_f[:, 44 - t])
                    else:
                        hT = mT  # v == 0: no incoming fwd edge

                # --- prefetch next step's psum inits ---
                if t + 1 < T:
                    un, vn = int(SRC[t + 1]), int(DST[t + 1])
                    nfresh = (t + 1) == L - 1
                    psz = ps_z.tile([BL, H], F32, tag="z")
                    psh = ps_h.tile([BL, H], F32, tag="h")
                    nc.tensor.matmul(psz[:], ident_b[:], Gall[:, un, 0:H],
                                     start=True, stop=nfresh)
                    nc.tensor.matmul(psh[:], ident_b[:], Gall[:, un, H:2 * H],
                                     start=True, stop=nfresh)
                    if not (t + 1 == L - 2 or t + 1 == T - 1):
                        psr = ps_r.tile([BL, H], F32, tag="r")
                        nc.tensor.matmul(psr[:], ident_b[:], Gall[:, vn, 2 * H:3 * H],
                                         start=True, stop=False)

                # ---------------- interleaved tail work ----------------
                # q pipeline first (releases its pq psum before p allocates):
                # advance post stage (older rows), then mid, then new mms
                nxt = []
                for item in pending_q:
                    if item[0] == "post":
                        if DBG_Q_STAGE >= 3:
                            emit_q_post(item[1], item[2])
                    elif item[0] == "mid":
                        if DBG_Q_STAGE >= 2:
                            pslog = emit_q_mid(item[1])
                            nxt.append(("post", pslog, item[2]))
                pending_q = nxt
                if not DBG_NO_Q:
                    for k in qrow_at_step.get(t, []):
                        if k == 0:
                            psq = emit_q_mm(None, 0)
                        else:
                            psq = emit_q_mm(hsT_f[:, k - 1], k)
                        pending_q.append(("mid", psq, k))

                # p-row for PREVIOUS step (hT_last); root p-row at t==0 (h=None)
                if DBG_NO_P:
                    pass
                elif t == 0:
                    psp = emit_p_mm(None, 0)
                    pending_p.append((psp, 0))
                else:
                    psp = emit_p_mm(hT_last, int(DST[t - 1]))
                    pending_p.append((psp, t))
                # drain p pipeline with one-step delay
                while len(pending_p) > 1:
                    psp_, row_ = pending_p.pop(0)
                    if DBG_P_STAGE >= 2:
                        emit_p_post(psp_, row_)

                if t == T - 1:
                    # flush remaining pipeline work
                    while pending_q:
                        nxt = []
                        for item in pending_q:
                            if item[0] == "post":
                                if DBG_Q_STAGE >= 3:
                                    emit_q_post(item[1], item[2])
                            elif item[0] == "mid":
                                if DBG_Q_STAGE >= 2:
                                    pslog = emit_q_mid(item[1])
                                    nxt.append(("post", pslog, item[2]))
                        pending_q = nxt
                    if not DBG_NO_P:
                        psp_, row_ = pending_p.pop(0)
                        if DBG_P_STAGE >= 2:
                            emit_p_post(psp_, row_)
                        psp = emit_p_mm(hT, v)
                        if DBG_P_STAGE >= 2:
                            emit_p_post(psp, T)

                mT_prev, rmT_prev = mT, rmT
                s_bf = m_bf
                hT_last = hT

            # ---------------- batched losses ----------------
            if DBG_NO_P:
                nc.vector.memset(pvals[:], 0.0)
            if DBG_NO_Q:
                nc.vector.memset(q_S[:], 1.0)
                nc.vector.memset(q_mx[:], 0.0)
                nc.vector.memset(q_tg[:], 0.0)
            # p: pvals += bs ; ploss_row = ln(1+exp(p)) - p*tgt ; pacc = ((p>0)==tgt)
            nc.vector.tensor_scalar(pvals[:], pvals[:], bs_b[:, 0:1], None, op0=ALU.add)
            pm = rgs.tile([BL, T + 1], F32, tag="pm")
            nc.vector.memset(pm[:, :L - 1], 1.0)
            nc.vector.memset(pm[:, L - 1:], 0.0)
            pe = rgs.tile([BL, T + 1], F32, tag="pe")
            nc.scalar.activation(pe[:], pvals[:], AF.Exp)
            nc.vector.tensor_scalar_add(pe[:], pe[:], 1.0)
            sp = rgs.tile([BL, T + 1], F32, tag="spl")
            nc.scalar.activation(sp[:], pe[:], AF.Ln)
            ptd = rgs.tile([BL, T + 1], F32, tag="ptd")
            nc.vector.tensor_mul(ptd[:], pvals[:], pm[:])
            nc.vector.tensor_sub(ptd[:], sp[:], ptd[:])
            acc = pp.tile([BL, 4], F32, tag="acc")  # qloss, ploss, qacc, pacc
            nc.vector.tensor_reduce(acc[:, 1:2], ptd[:], axis=mybir.AxisListType.X,
                                    op=ALU.add)
            pg = rgs.tile([BL, T + 1], F32, tag="pg")
            nc.vector.tensor_scalar(pg[:], pvals[:], 0.0, None, op0=ALU.is_gt)
            nc.vector.tensor_tensor(pg[:], pg[:], pm[:], ALU.is_equal)
            nc.vector.tensor_reduce(acc[:, 3:4], pg[:], axis=mybir.AxisListType.X,
                                    op=ALU.add)
            # q: qloss_row = ln(S) - q[tgt] ; qacc_row = (q[tgt] == max)
            lns = rgs.tile([BL, L], F32, tag="lns")
            nc.scalar.activation(lns[:], q_S[:], AF.Ln)
            nc.vector.tensor_sub(lns[:], lns[:], q_tg[:])
            nc.vector.tensor_reduce(acc[:, 0:1], lns[:], axis=mybir.AxisListType.X,
                                    op=ALU.add)
            qe = rgs.tile([BL, L], F32, tag="qe")
            nc.vector.tensor_tensor(qe[:], q_tg[:], q_mx[:], ALU.is_equal)
            nc.vector.tensor_reduce(acc[:, 2:3], qe[:], axis=mybir.AxisListType.X,
                                    op=ALU.add)

            # ---------------- final cross-partition reduce ----------------
            ones = rgs.tile([128, 1], F32, tag="ones")
            nc.vector.memset(ones[:], 1.0)
            psf = ps_pq.tile([BL, H], F32, tag="pq")
            nc.tensor.matmul(psf[:1, :4], ones[:], acc[:], start=True, stop=True)
            outs = rgs.tile([1, 4], F32, tag="outs")
            nc.scalar.copy(outs[:], psf[:1, :4])
            nc.sync.dma_start(d_out[:], outs[:])

    nc.compile()
    return nc


_CACHE = {}


def _prep_inputs(wid, tree_vec, emb, Wz, bz, Wr, Ur, bur, Wh, bh, Ww, bw, Wu, bu,
                 Wo, bo, Us, bs):
    emb64 = emb.astype(np.float64)
    Pz = (emb64 @ Wz[:H].astype(np.float64) + bz) * 0.5   # tanh-form: /2
    Ph = emb64 @ Wh[:H].astype(np.float64) + bh
    Pr = (emb64 @ Wr.astype(np.float64) + bur) * 0.5
    Pu = emb64 @ Wu[:H].astype(np.float64) + bu
    P = np.concatenate([Pz, Ph, Pr, Pu], axis=1).astype(_BF)

    def wlay(w):  # [K, N] -> [128, K/128, N]
        return np.ascontiguousarray(
            np.asarray(w).reshape(w.shape[0] // 128, 128, w.shape[1]).transpose(1, 0, 2)
        ).astype(_BF)

    shared = dict(
        P=P,
        # z-pre and r-pre are halved (tanh form).  rm' = m + w*m = 2*rm, so
        # Whm absorbs another /2; Ur absorbs rm'... (note: arm' = 2*arm, and
        # h-pre needs arm@Whm -> (arm')@(Whm/2)).  r-pre = (dst@Wr + m@Ur +
        # bur)/2 -> Ur/2 with the table already halved.
        Wzs=wlay(Wz[H:].astype(np.float64) * 0.5),
        Whm=wlay(Wh[H:].astype(np.float64) * 0.5),
        Ur=wlay(Ur.astype(np.float64) * 0.5),
        Uh=wlay(Wu[H:2 * H]),
        Wwh=wlay(Ww[:H]),
        Wo=wlay(Wo),
        Wutv=np.ascontiguousarray(Wu[2 * H:]).astype(np.float32),
        Wwtv=np.ascontiguousarray(Ww[H:]).astype(np.float32),
        bw=np.asarray(bw)[None, :].astype(np.float32),
        us=np.asarray(Us)[:, 0][None, :].astype(np.float32),
        bs=np.asarray(bs).reshape(1, 1).astype(np.float32),
        bo=np.asarray(bo)[None, :].astype(np.float32),
    )
    in_maps = []
    for c in range(NCORES):
        sl = slice(c * BL, (c + 1) * BL)
        m = dict(shared)
        m["wid"] = np.ascontiguousarray(wid[sl]).astype(np.int32)
        m["tv"] = np.ascontiguousarray(tree_vec[sl]).astype(np.float32)
        in_maps.append(m)
    return in_maps


def kernel(**inputs):
    npa = {k: np.asarray(v) for k, v in inputs.items()}
    if "nc" not in _CACHE:
        _CACHE["nc"] = _build_nc()
    nc = _CACHE["nc"]
    in_maps = _prep_inputs(**npa)
    res = run_bass_kernel_spmd(nc, in_maps, core_ids=list(range(NCORES)))
    _CACHE["last_res"] = res
    tot = np.zeros(4, dtype=np.float64)
    for r in res.results:
        tot += r["out"][0].astype(np.float64)
    q_loss = tot[0] / B
    p_loss = tot[1] / B
    q_acc = tot[2] / (L * B)
    p_acc = tot[3] / ((T + 1) * B)
    return np.array([q_loss, p_loss, q_acc, p_acc], dtype=np.float32)
